# revision 31
# baseline (speedup 1.0000x reference)
"""GlobalPointer-style head (RoPE'd QK^T with pad + strict-lower-tri masks).

Self-contained Trainium2 Bass kernel. Accepts FULL inputs, shards batch 16 ->
8 cores (2 per core), runs one SPMD Bass program, gathers FULL output.

Math per (b, h):
  qk = x[b].reshape(512, 12, 128); q = qk[..., :64], k = qk[..., 64:]
  q' = (q*cos + rot(q)*sin) / 8        (1/8 folded into q cos/sin tables)
  k' = (k*cos + rot(k)*sin)
  out[b,h] = q' @ k'^T  masked to -NEG/8 on the strict lower triangle
(the attention mask is all-ones for this problem's inputs; a general-mask
fallback kernel is compiled lazily if a non-ones mask is ever passed).

Fast-path structure per core (2 batches):
  - load x chunks (128 rows), cast fp32->bf16 on ACT, RoPE in bf16 on DVE
  - per (chunk c, head-pair g): two PE transposes with strided stationary
    APs put [qT_h2g | qT_h2g+1] and [kT_h2g | kT_h2g+1] at partitions
    0-63 / 64-127 of one PSUM tile; one ACT copy stages them to SBUF
  - per (head h, chunk c): ONE bf16 matmul (K=64) computing only the live
    column suffix n >= c*128 into fp32 PSUM
  - epilogue: DVE tensor_add fuses the strict-lower diagonal-block bias with
    the PSUM->SBUF move; ACT copies the bias-free remainder
  - the fully-masked column prefix of each output row block is a constant
    (-NEG/8), pre-filled ONCE into the rotating output buffers
  - one output DMA per (b, h): 512 descriptors x 2KB contiguous rows
"""

import sys

import numpy as np

for _p in ("/opt/trn_rl_repo",):
    if _p not in sys.path:
        sys.path.insert(0, _p)

import ml_dtypes  # noqa: E402

import concourse.mybir as mybir  # noqa: E402
import concourse.tile as tile  # noqa: E402
from concourse import bacc  # noqa: E402
from concourse.bass_utils import run_bass_kernel_spmd  # noqa: E402
from concourse.masks import make_identity  # noqa: E402

F32 = mybir.dt.float32
BF16 = mybir.dt.bfloat16

N_CORES = 8
B, M, H, D = 16, 512, 12, 64
BS = B // N_CORES  # batches per core
MC = M // 128  # m-chunks of 128
G = H // 2  # head pairs
NEG = np.float32(1.0e12)
V8 = np.float32(NEG / np.float32(8.0))  # 1.25e11, exact in fp32
OSB_BUFS = 6


def _tables():
    """Host-precomputed constants (functions of position only, not of data)."""
    pos = np.arange(M, dtype=np.float32)[:, None]
    inv_freq = np.power(
        np.float32(10000.0),
        (np.float32(-2.0) * np.arange(D // 2, dtype=np.float32) / np.float32(D)),
    )
    ang = pos * inv_freq[None, :]  # (M, 32)
    cos = np.repeat(np.cos(ang), 2, axis=1)  # (M, 64)
    sin = np.repeat(np.sin(ang), 2, axis=1)  # (M, 64)
    sign = np.where(np.arange(D) % 2 == 0, np.float32(-1.0), np.float32(1.0))
    sin_signed = sin * sign[None, :]

    scale = np.float32(1.0 / 8.0)
    cos_t = np.concatenate([cos * scale, cos], axis=1).astype(np.float32)
    sin_t = np.concatenate([sin_signed * scale, sin_signed], axis=1).astype(np.float32)
    cos_b = cos_t.astype(ml_dtypes.bfloat16)
    sin_b = sin_t.astype(ml_dtypes.bfloat16)

    # strict-lower 128x128 diagonal block padded with zero cols to 512, fp32
    p = np.arange(128)
    tdiag = np.zeros((128, 512), dtype=np.float32)
    tdiag[:, :128] = np.where(p[:, None] > p[None, :], -V8, np.float32(0.0))

    return cos_b, sin_b, tdiag


def build_fast():
    nc = bacc.Bacc("TRN2", target_bir_lowering=False, debug=False)

    x_d = nc.dram_tensor("x", [BS, M, H * 2 * D], F32, kind="ExternalInput")
    cos_d = nc.dram_tensor("cos_t", [M, 2 * D], BF16, kind="ExternalInput")
    sin_d = nc.dram_tensor("sin_t", [M, 2 * D], BF16, kind="ExternalInput")
    tdiag_d = nc.dram_tensor("tdiag", [128, 512], F32, kind="ExternalInput")
    out_d = nc.dram_tensor("out", [BS, H, M, M], F32, kind="ExternalOutput")

    with tile.TileContext(nc) as tc:
        with (
            tc.tile_pool(name="const", bufs=1) as cpool,
            tc.tile_pool(name="xin", bufs=2) as xpool,
            tc.tile_pool(name="rope", bufs=2) as rpool,
            tc.tile_pool(name="qk", bufs=2) as qkpool,
            tc.tile_pool(name="ps_t", bufs=2, space="PSUM") as pst_pool,
            tc.tile_pool(name="ps_a", bufs=1, space="PSUM") as psa_pool,
            tc.tile_pool(name="ps_b", bufs=1, space="PSUM") as psb_pool,
            tc.tile_pool(name="ps_c", bufs=1, space="PSUM") as psc_pool,
        ):
            ident = cpool.tile([128, 128], BF16)
            make_identity(nc, ident)

            # ---- first x chunk, then the small tables, then remaining x
            # chunks: the sync DMA queue is in-order, so this gets cast+RoPE
            # of chunk 0 started as early as possible
            xf = []
            t = xpool.tile([128, H * 2 * D], F32, name="x00", tag="xn", bufs=2 * MC)
            nc.sync.dma_start(out=t[:], in_=x_d[0, 0:128, :])
            xf.append(t)

            cos_sb = cpool.tile([128, MC * 2 * D], BF16)
            nc.sync.dma_start(
                out=cos_sb[:].rearrange("p (c f) -> p c f", c=MC),
                in_=cos_d[:].rearrange("(c p) f -> p c f", p=128),
            )
            sin_sb = cpool.tile([128, MC * 2 * D], BF16)
            nc.sync.dma_start(
                out=sin_sb[:].rearrange("p (c f) -> p c f", c=MC),
                in_=sin_d[:].rearrange("(c p) f -> p c f", p=128),
            )
            tdiag_sb = cpool.tile([128, 512], F32)
            nc.sync.dma_start(out=tdiag_sb[:], in_=tdiag_d[:])

            for b in range(BS):
                for c in range(MC):
                    if b == 0 and c == 0:
                        continue
                    t = xpool.tile(
                        [128, H * 2 * D], F32, name=f"x{b}{c}", tag="xn", bufs=2 * MC
                    )
                    nc.sync.dma_start(out=t[:], in_=x_d[b, c * 128 : (c + 1) * 128, :])
                    xf.append(t)

            # persistent output staging buffers; masked column-prefix constants
            # are filled once and survive round-robin reuse
            osb = [
                cpool.tile([128, MC * M], F32, name=f"osb{j}", tag=f"osb{j}")
                for j in range(OSB_BUFS)
            ]
            for t in osb:
                for c in range(1, MC):
                    nc.gpsimd.memset(t[:, c * M : c * M + c * 128], float(-V8))

            # ---- masked-prefix constants of row-chunks 2 and 3 of the output:
            # written from the pre-filled osb[0] prefix, broadcast over heads,
            # on the otherwise-idle GPSIMD DMA queue. ~22us of filler work the
            # DMA engines pull in whenever the live-output stream has a
            # bubble (pipeline rampup, the batch0->batch1 transpose phase).
            # The tiny copy below makes the GPSIMD queue wait for the last
            # x-load first, so this filler doesn't compete with the input
            # loads on the DMA engines.
            defer = cpool.tile([1, 4], F32)
            nc.gpsimd.tensor_copy(out=defer[:], in_=xf[-1][0:1, 0:4])
            for b in range(BS):
                for c in (3,):
                    w = c * 128
                    src = (
                        osb[0][:, c * M : c * M + w]
                        .rearrange("p (o n) -> p o n", o=1)
                        .to_broadcast((128, H, w))
                    )
                    dst = out_d[
                        b, :, c * 128 : (c + 1) * 128, 0:w
                    ].rearrange("h p n -> p h n")
                    nc.gpsimd.dma_start(out=dst, in_=src)

            # ---- cast (ACT) for both batches up front; RoPE (DVE) only for
            # batch 0 here. Batch 1's RoPE is interleaved between batch 0's
            # head pairs below, so the in-order DVE queue doesn't make batch
            # 0's epilogue adds wait behind 11us of batch-1 RoPE.
            xr = [[None] * MC for _ in range(BS)]
            xbs = [[None] * MC for _ in range(BS)]
            for b in range(BS):
                for c in range(MC):
                    src = xf[b * MC + c]
                    xb = xpool.tile([128, H * 2 * D], BF16, tag="xb", bufs=2 * MC)
                    nc.scalar.copy(out=xb[:], in_=src[:])
                    xbs[b][c] = xb

            def emit_rope(b, c):
                xb = xbs[b][c]
                t1 = rpool.tile([128, H * 2 * D], BF16, tag="t1", bufs=2)
                t2 = rpool.tile([128, H * 2 * D], BF16, tag="t2", bufs=2)
                xr_c = rpool.tile([128, H * 2 * D], BF16, tag="xr", bufs=2 * MC)
                src4 = xb[:].rearrange("p (h a two) -> p h a two", two=2, a=D)
                swp4 = src4[:, :, :, ::-1]
                t14 = t1[:].rearrange("p (h a two) -> p h a two", two=2, a=D)
                t24 = t2[:].rearrange("p (h a two) -> p h a two", two=2, a=D)
                xr4 = xr_c[:].rearrange("p (h a two) -> p h a two", two=2, a=D)
                cs = slice(c * 2 * D, (c + 1) * 2 * D)
                cos_v = (
                    cos_sb[:, cs]
                    .rearrange("p (o a two) -> p o a two", o=1, two=2)
                    .to_broadcast((128, H, D, 2))
                )
                sin_v = (
                    sin_sb[:, cs]
                    .rearrange("p (o a two) -> p o a two", o=1, two=2)
                    .to_broadcast((128, H, D, 2))
                )
                nc.vector.tensor_mul(out=t14, in0=swp4, in1=sin_v)
                nc.vector.tensor_mul(out=t24, in0=src4, in1=cos_v)
                nc.vector.tensor_add(out=xr4, in0=t24, in1=t14)
                xr[b][c] = xr_c

            for c in range(MC):
                emit_rope(0, c)

            # ---- per (batch, head-pair): transposes, matmuls, epilogue,
            # store. Interleaving transposes with the matmul stream keeps
            # head production continuous so the output DMA never starves.
            obuf = 0
            for b in range(BS):
                # head-pair staging: [128, MC*256] layout (c, {qT|kT}, 128),
                # partitions 0-63 = even head, 64-127 = odd head.
                # Transposes are regular matmuls by identity, fp32 PSUM out,
                # column-tiled 128x64 so even/odd heads run concurrently on
                # tiles (0,0)/(0,64); cast to bf16 in the ACT staging copy.
                qk = [
                    qkpool.tile([128, MC * 256], BF16, name=f"qkg{g}", tag=f"qk{g}")
                    for g in range(G)
                ]
                for g in range(G):
                    # slot batch 1's RoPE chunks between batch 0's early head
                    # pairs on the DVE queue
                    if b == 0 and g < MC:
                        emit_rope(1, g)
                    for c in range(MC):
                        xv = xr[b][c]
                        pst = pst_pool.tile([128, 256], F32, tag="pst")
                        for h2 in range(2):
                            hc = (2 * g + h2) * 128
                            pr = slice(64 * h2, 64 * h2 + 64)
                            nc.tensor.matmul(
                                pst[pr, 0:128],
                                xv[:, hc : hc + 64],
                                ident[:],
                                start=True,
                                stop=True,
                            )
                            nc.tensor.matmul(
                                pst[pr, 128:256],
                                xv[:, hc + 64 : hc + 128],
                                ident[:],
                                start=True,
                                stop=True,
                            )
                        nc.scalar.copy(
                            out=qk[g][:, c * 256 : (c + 1) * 256], in_=pst[:]
                        )

                    # matmuls: row-tiled 64x128, even head on PE rows 0-63
                    # and odd head on rows 64-127 execute concurrently
                    ps = []
                    for h2 in range(2):
                        ps.append(
                            (
                                psa_pool.tile([128, 512], F32, name=f"psa{h2}", tag=f"psa{h2}"),
                                psb_pool.tile([128, 384], F32, name=f"psb{h2}", tag=f"psb{h2}"),
                                psc_pool.tile([128, 384], F32, name=f"psc{h2}", tag=f"psc{h2}"),
                            )
                        )
                    for c in range(MC):
                        for h2 in range(2):
                            prow = slice(64 * h2, 64 * h2 + 64)
                            kt3 = (
                                qk[g][prow, :]
                                .rearrange("p (c two n) -> p c two n", two=2, n=128)
                            )
                            ps_a, ps_b, ps_c = ps[h2]
                            dest = [
                                (ps_a, 0, 512),
                                (ps_b, 0, 384),
                                (ps_c, 0, 256),
                                (ps_c, 256, 128),
                            ][c]
                            pt, off, nlive = dest
                            nc.tensor.matmul(
                                pt[:, off : off + nlive],
                                qk[g][prow, c * 256 : c * 256 + 128],
                                kt3[:, c:, 1, :],
                                start=True,
                                stop=True,
                            )

                    for h2 in range(2):
                        h = 2 * g + h2
                        ps_a, ps_b, ps_c = ps[h2]
                        ob = osb[obuf]
                        obuf = (obuf + 1) % OSB_BUFS

                        # epilogue: diag-block bias fused into PSUM->SBUF move
                        # c=0: ACT copies cols 128:512, DVE adds tdiag on 0:128
                        nc.scalar.copy(out=ob[:, 128:512], in_=ps_a[:, 128:512])
                        nc.vector.tensor_add(
                            out=ob[:, 0:128],
                            in0=ps_a[:, 0:128],
                            in1=tdiag_sb[:, 0:128],
                        )
                        # c=1: one DVE add over the whole 384 live cols
                        nc.vector.tensor_add(
                            out=ob[:, 640:1024],
                            in0=ps_b[:, 0:384],
                            in1=tdiag_sb[:, 0:384],
                        )
                        # c=2: DVE diag add + ACT copy of cols 128:256
                        nc.vector.tensor_add(
                            out=ob[:, 1280:1408],
                            in0=ps_c[:, 0:128],
                            in1=tdiag_sb[:, 0:128],
                        )
                        nc.scalar.copy(out=ob[:, 1408:1536], in_=ps_c[:, 128:256])
                        # c=3: DVE diag add only
                        nc.vector.tensor_add(
                            out=ob[:, 1920:2048],
                            in0=ps_c[:, 256:384],
                            in1=tdiag_sb[:, 0:128],
                        )

                        # stores: full rows for chunks 0-2 (prefixes from the
                        # memset regions of osb), live-suffix only for chunk 3
                        # (its prefix was pre-written above)
                        nc.sync.dma_start(
                            out=out_d[b, h, 0 : 3 * 128, :].rearrange(
                                "(c p) n -> p c n", p=128
                            ),
                            in_=ob[:, 0 : 3 * M].rearrange("p (c n) -> p c n", c=3),
                        )
                        nc.sync.dma_start(
                            out=out_d[b, h, 3 * 128 : M, 3 * 128 : M],
                            in_=ob[:, 3 * M + 3 * 128 : 4 * M],
                        )

    nc.compile()
    return nc


def build_general():
    """Baseline kernel handling arbitrary attention masks (lazy fallback)."""
    nc = bacc.Bacc("TRN2", target_bir_lowering=False, debug=False)

    x_d = nc.dram_tensor("x", [BS, M, H * 2 * D], F32, kind="ExternalInput")
    mask_d = nc.dram_tensor("mask", [BS, M], F32, kind="ExternalInput")
    cos_d = nc.dram_tensor("cos_t", [M, 2 * D], BF16, kind="ExternalInput")
    sin_d = nc.dram_tensor("sin_t", [M, 2 * D], BF16, kind="ExternalInput")
    tdiag_d = nc.dram_tensor("tdiag", [128, 512], F32, kind="ExternalInput")
    out_d = nc.dram_tensor("out", [BS, H, M, M], F32, kind="ExternalOutput")

    mult = mybir.AluOpType.mult
    add = mybir.AluOpType.add

    with tile.TileContext(nc) as tc:
        with (
            tc.tile_pool(name="const", bufs=1) as cpool,
            tc.tile_pool(name="xin", bufs=2) as xpool,
            tc.tile_pool(name="rope", bufs=2) as rpool,
            tc.tile_pool(name="small", bufs=2) as spool,
            tc.tile_pool(name="xt", bufs=3) as tpool,
            tc.tile_pool(name="osb", bufs=3) as opool,
            tc.tile_pool(name="ps_t", bufs=3, space="PSUM") as pst_pool,
            tc.tile_pool(name="ps_mm", bufs=2, space="PSUM") as psm_pool,
        ):
            ident = cpool.tile([128, 128], BF16)
            make_identity(nc, ident)
            ones_row = cpool.tile([1, 128], BF16)
            nc.gpsimd.memset(ones_row[:], 1.0)
            ones_f32 = cpool.tile([1, 128], F32)
            nc.gpsimd.memset(ones_f32[:], 1.0)

            cos_sb = cpool.tile([128, MC * 2 * D], BF16)
            nc.sync.dma_start(
                out=cos_sb[:].rearrange("p (c f) -> p c f", c=MC),
                in_=cos_d[:].rearrange("(c p) f -> p c f", p=128),
            )
            sin_sb = cpool.tile([128, MC * 2 * D], BF16)
            nc.sync.dma_start(
                out=sin_sb[:].rearrange("p (c f) -> p c f", c=MC),
                in_=sin_d[:].rearrange("(c p) f -> p c f", p=128),
            )
            tdiag_sb = cpool.tile([128, 128], F32)
            nc.sync.dma_start(out=tdiag_sb[:], in_=tdiag_d[:, 0:128])

            copy_rr = 0
            for b in range(BS):
                xb = []
                for c in range(MC):
                    t = xpool.tile([128, H * 2 * D], F32, tag="xn", bufs=4)
                    nc.sync.dma_start(out=t[:], in_=x_d[b, c * 128 : (c + 1) * 128, :])
                    tb = xpool.tile([128, H * 2 * D], BF16, tag="xb", bufs=8)
                    nc.gpsimd.tensor_copy(out=tb[:], in_=t[:])
                    xb.append(tb)

                padrow = spool.tile([1, M], F32, tag="padrow")
                nc.sync.dma_start(out=padrow[:], in_=mask_d[b : b + 1, :])
                padbias = spool.tile([1, M], F32, tag="padbias")
                nc.vector.tensor_scalar(
                    out=padbias[:],
                    in0=padrow[:],
                    scalar1=float(V8),
                    scalar2=float(-V8),
                    op0=mult,
                    op1=add,
                )
                colb = spool.tile([1, M], BF16, tag="colb")
                nc.gpsimd.tensor_copy(out=colb[:], in_=padbias[:])
                rowvals = spool.tile([1, M], F32, tag="rowvals")
                nc.vector.tensor_scalar(
                    out=rowvals[:],
                    in0=padrow[:],
                    scalar1=float(V8),
                    scalar2=float(-2.0 * V8),
                    op0=mult,
                    op1=add,
                )
                ps_cf = pst_pool.tile([128, M], F32, tag="pscf", bufs=1)
                nc.tensor.matmul(
                    ps_cf[:], ones_f32[:], rowvals[:], start=True, stop=True
                )
                colfull = spool.tile([128, M], F32, tag="colfull")
                nc.vector.tensor_copy(out=colfull[:], in_=ps_cf[:])

                padcol = spool.tile([128, MC], F32, tag="padcol")
                nc.sync.dma_start(
                    out=padcol[:], in_=mask_d[b, :].rearrange("(c p) -> p c", p=128)
                )

                xr = []
                for c in range(MC):
                    src = xb[c]
                    t1 = rpool.tile([128, H * 2 * D], BF16, tag="t1", bufs=2)
                    t2 = rpool.tile([128, H * 2 * D], BF16, tag="t2", bufs=2)
                    xr_c = rpool.tile([128, H * 2 * D], BF16, tag="xr", bufs=8)
                    src4 = src[:].rearrange("p (h a two) -> p h a two", two=2, a=D)
                    swp4 = src4[:, :, :, ::-1]
                    t14 = t1[:].rearrange("p (h a two) -> p h a two", two=2, a=D)
                    t24 = t2[:].rearrange("p (h a two) -> p h a two", two=2, a=D)
                    xr4 = xr_c[:].rearrange("p (h a two) -> p h a two", two=2, a=D)
                    cs = slice(c * 2 * D, (c + 1) * 2 * D)
                    cos_v = (
                        cos_sb[:, cs]
                        .rearrange("p (o a two) -> p o a two", o=1, two=2)
                        .to_broadcast((128, H, D, 2))
                    )
                    sin_v = (
                        sin_sb[:, cs]
                        .rearrange("p (o a two) -> p o a two", o=1, two=2)
                        .to_broadcast((128, H, D, 2))
                    )
                    nc.vector.tensor_mul(out=t14, in0=swp4, in1=sin_v)
                    nc.vector.tensor_mul(out=t24, in0=src4, in1=cos_v)
                    nc.vector.tensor_add(out=xr4, in0=t24, in1=t14)
                    k3 = xr_c[:].rearrange("p (h f) -> p h f", f=2 * D)[:, :, D:]
                    nc.vector.tensor_scalar(
                        out=k3,
                        in0=k3,
                        scalar1=padcol[:, c : c + 1],
                        scalar2=None,
                        op0=mult,
                    )
                    xr.append(xr_c)

                for h in range(H):
                    qkt = tpool.tile([D, MC * 2 * 128], BF16, tag="qkt")
                    for c in range(MC):
                        ps_t = pst_pool.tile([D, 256], BF16, tag="pst", bufs=3)
                        nc.tensor.transpose(
                            ps_t[:, 0:128],
                            xr[c][:, h * 2 * D : h * 2 * D + D],
                            ident[:],
                        )
                        nc.tensor.transpose(
                            ps_t[:, 128:256],
                            xr[c][:, h * 2 * D + D : (h + 1) * 2 * D],
                            ident[:],
                        )
                        nc.scalar.copy(
                            out=qkt[:, c * 256 : (c + 1) * 256], in_=ps_t[:]
                        )
                    qkt3 = qkt[:].rearrange("p (c two f) -> p c two f", two=2, f=128)
                    kt_ap = qkt3[:, :, 1, :]
                    osb = opool.tile([128, MC * M], F32, tag="osb")
                    for g in range(2):
                        ps_mm = psm_pool.tile([128, 2 * M], F32, tag="psmm", bufs=2)
                        for cc in range(2):
                            c = g * 2 + cc
                            nc.tensor.matmul(
                                ps_mm[:, cc * M : (cc + 1) * M],
                                qkt[:, c * 256 : c * 256 + 128],
                                kt_ap,
                                start=True,
                                stop=False,
                            )
                            nc.tensor.matmul(
                                ps_mm[:, cc * M : (cc + 1) * M],
                                ones_row[:],
                                colb[0:1, :],
                                start=False,
                                stop=True,
                            )
                        for cc in range(2):
                            c = g * 2 + cc
                            lw = c * 128
                            use_act = (copy_rr % 8) < 5
                            copy_rr += 1
                            if lw:
                                nc.gpsimd.tensor_copy(
                                    out=osb[:, c * M : c * M + lw],
                                    in_=colfull[:, 0:lw],
                                )
                            if use_act:
                                nc.scalar.copy(
                                    out=osb[:, c * M + lw : (c + 1) * M],
                                    in_=ps_mm[:, cc * M + lw : (cc + 1) * M],
                                )
                            else:
                                nc.vector.tensor_copy(
                                    out=osb[:, c * M + lw : (c + 1) * M],
                                    in_=ps_mm[:, cc * M + lw : (cc + 1) * M],
                                )
                            ds = slice(c * M + lw, c * M + lw + 128)
                            nc.vector.tensor_add(
                                out=osb[:, ds], in0=osb[:, ds], in1=tdiag_sb[:]
                            )
                    nc.sync.dma_start(
                        out=out_d[b, h].rearrange("(c p) n -> p c n", p=128),
                        in_=osb[:].rearrange("p (c n) -> p c n", c=MC),
                    )

    nc.compile()
    return nc


_NC_FAST = None
_NC_GEN = None
_TABLES = None


def _get_tables():
    global _TABLES
    if _TABLES is None:
        _TABLES = _tables()
    return _TABLES


def _get_fast():
    global _NC_FAST
    if _NC_FAST is None:
        _NC_FAST = build_fast()
    return _NC_FAST


def _get_general():
    global _NC_GEN
    if _NC_GEN is None:
        _NC_GEN = build_general()
    return _NC_GEN


def run(x, attention_mask, **run_kwargs):
    cos_b, sin_b, tdiag = _get_tables()
    x = np.ascontiguousarray(np.asarray(x, dtype=np.float32))
    am = np.ascontiguousarray(np.asarray(attention_mask, dtype=np.float32))
    fast = bool(np.all(am == np.float32(1.0)))
    nc = _get_fast() if fast else _get_general()
    maps = []
    for i in range(N_CORES):
        sl = slice(i * BS, (i + 1) * BS)
        m = {
            "x": np.ascontiguousarray(x[sl]),
            "cos_t": cos_b,
            "sin_t": sin_b,
            "tdiag": tdiag,
        }
        if not fast:
            m["mask"] = np.ascontiguousarray(am[sl])
        maps.append(m)
    res = run_bass_kernel_spmd(nc, maps, list(range(N_CORES)), **run_kwargs)
    out = np.concatenate([r["out"] for r in res.results], axis=0)
    return out, res


def kernel(x, attention_mask, token_type_ids=None, **_unused):
    out, _ = run(x, attention_mask)
    return out
